# revision 3
# baseline (speedup 1.0000x reference)
"""GIN decoder (segment-sum + 2-layer MLP) on 8 trn2 NeuronCores — fused v2.

Data-parallel over dst nodes (2500/core).  One fused tile program per core:

  - Aggregation: edges host-bucketed by (core, 128-wide dst block) into
    128-edge tiles; each tile gathers x rows by src id with one 2-D indirect
    DMA [128, 512] (per-row 1 KB descriptors — 3-D/batched offset forms are
    pathologically slow or wrong on HW) and scatter-adds within the block
    via a one-hot matmul accumulated in PSUM.  The GIN self term x_i rides
    the PSUM eviction from a direct (non-gathered) x slice; the block is
    PE-transposed to feature-major h^T held in SBUF.
  - MLP: h^T and h1^T never touch DRAM.  Nodes run in 5 growing col-slices
    (2/3/4/5/6 blocks); W2 streams from DRAM once per slice.  h1^T of a
    slice stays SBUF-resident while its mm2 runs; the NEXT slice's
    aggregation is interleaved into the mm2 instruction stream with a
    48-deep gather ring so the Pool-engine-serial indirect DMAs (~2.9 us
    per 128-row call — the real bottleneck on HW) hide under the matmul.
  - Output written f32 as out^T [128, 64, 2500] per core; host unpacks.

All matmuls bf16 with f32 PSUM accumulation.
"""

import math

import numpy as np
import ml_dtypes

P = 128
N_NODES = 20000
HIDDEN = 512
MIDDLE = 4352
VOCAB = 8192
NCORES = 8
ND = N_NODES // NCORES          # 2500 nodes per core
NBLK = 20                       # dst blocks of 128 (last block 68 valid)
# Bump VERSION on every device-program change: the jax/neuronxcc compile
# cache keys on tensor shapes only, so x_rows row padding salts the key.
VERSION = 5
KR = N_NODES + 1 + VERSION      # x rows + zero row for pad edges + salt
ZERO_ROW = N_NODES
KSUB1 = HIDDEN // P             # 4
M1 = MIDDLE // P                # 34
M2 = VOCAB // P                 # 64
GC = 16                         # max gather cols per indirect DMA call
BF16 = ml_dtypes.bfloat16

# col slices: (block_start, nblocks, valid_width).  Widths grow so that each
# slice's mm2 window covers the (Pool-engine-serial) gather time of the next
# slice's aggregation; the small first slice keeps the gather-bound prologue
# short.
SLICES = ((0, 2, 256), (2, 3, 384), (5, 4, 512), (9, 5, 640), (14, 6, 708))


def _chunks(width):
    out = []
    n0 = 0
    while n0 < width:
        w = min(512, width - n0)
        out.append((n0, w))
        n0 += w
    return tuple(out)


_BUILT = {}
LAST_RESULTS = None


def _emit_ablock(nc, ctx, st, b, bloc, h_tile, Tb, off):
    """Aggregation for dst block `b` (global), writing h_tile cols bloc*128."""
    import concourse.bass as bass
    from concourse import mybir

    dt = mybir.dt
    T = Tb[b]
    xb = st["xbp"].tile([P, HIDDEN], dt.bfloat16, name="xb")
    nc.sync.dma_start(xb[:], st["xself"][b])
    ps = st["aggps"].tile([P, HIDDEN], dt.float32, space="PSUM", name="aggps")
    for t in range(T):
        g = st["gp"].tile([P, HIDDEN], dt.bfloat16, name="g")
        nc.gpsimd.indirect_dma_start(
            out=g[:], out_offset=None, in_=st["x_rows"][:],
            in_offset=bass.IndirectOffsetOnAxis(
                ap=st["sid_sb"][:, off + t:off + t + 1], axis=0))
        oh = st["ohp"].tile([P, P], dt.bfloat16, name="oh")
        nc.vector.tensor_tensor(
            out=oh[:], in0=st["did_sb"][:, off + t:off + t + 1].to_broadcast([P, P]),
            in1=st["colidx_sb"][:], op=mybir.AluOpType.is_equal)
        nc.tensor.matmul(ps[:], lhsT=oh[:], rhs=g[:],
                         start=(t == 0), stop=(t == T - 1))
    hsb = st["hsbp"].tile([P, HIDDEN], dt.float32, name="hsb")
    nc.vector.tensor_tensor(out=hsb[:], in0=ps[:], in1=xb[:],
                            op=mybir.AluOpType.add)
    for j in range(KSUB1):
        tp = st["tpps"].tile([P, P], dt.float32, space="PSUM", name="tpps")
        nc.tensor.transpose(out=tp[:], in_=hsb[:, j * P:(j + 1) * P],
                            identity=st["ident"][:])
        nc.vector.tensor_copy(
            h_tile[:, j, bloc * P:(bloc + 1) * P], tp[:])


def _emit_mm1(nc, st, h_tile, h1_tile, chunks):
    from concourse import mybir
    dt = mybir.dt
    for m in range(M1):
        w1t = st["w1p_sb"].tile([P, KSUB1, P], dt.bfloat16, name="w1t")
        nc.sync.dma_start(w1t[:], st["w1p"][:, m])
        pss = [st["mmps"].tile([P, 512], dt.float32, space="PSUM",
                               name="mmps") for _ in chunks]
        for k in range(KSUB1):
            for ci, (n0, w) in enumerate(chunks):
                nc.tensor.matmul(pss[ci][:, :w], lhsT=w1t[:, k, :],
                                 rhs=h_tile[:, k, n0:n0 + w],
                                 start=(k == 0), stop=(k == KSUB1 - 1))
        for ci, (n0, w) in enumerate(chunks):
            nc.scalar.activation(
                h1_tile[:, m, n0:n0 + w], pss[ci][:, :w],
                mybir.ActivationFunctionType.Identity,
                bias=st["b1_sb"][:, m:m + 1])


def _emit_bunit(nc, st, h1_tile, m, chunks, coff):
    from concourse import mybir
    dt = mybir.dt
    w2t = st["w2p_sb"].tile([P, M1, P], dt.bfloat16, name="w2t")
    nc.sync.dma_start(w2t[:], st["w2p"][:, m])
    pss = [st["mmps"].tile([P, 512], dt.float32, space="PSUM", name="mmps")
           for _ in chunks]
    for k in range(M1):
        for ci, (n0, w) in enumerate(chunks):
            nc.tensor.matmul(pss[ci][:, :w], lhsT=w2t[:, k, :],
                             rhs=h1_tile[:, k, n0:n0 + w],
                             start=(k == 0), stop=(k == M1 - 1))
    for ci, (n0, w) in enumerate(chunks):
        ev = st["evp"].tile([P, 512], dt.float32, name="ev")
        nc.scalar.activation(ev[:, :w], pss[ci][:, :w],
                             mybir.ActivationFunctionType.Identity,
                             bias=st["b2_sb"][:, m:m + 1])
        nc.sync.dma_start(st["out_d"][:, m, coff + n0:coff + n0 + w],
                          ev[:, :w])


def _build(Tb):
    key = ("nc", Tb)
    if key in _BUILT:
        return _BUILT[key]
    from contextlib import ExitStack
    from concourse import bacc, mybir
    import concourse.tile as tile
    from concourse.masks import make_identity

    dt = mybir.dt
    sumT = int(sum(Tb))
    offs = np.concatenate([[0], np.cumsum(Tb)]).astype(int)

    nc = bacc.Bacc("TRN2", target_bir_lowering=False, debug=False,
                   num_devices=NCORES)

    x_rows = nc.dram_tensor("x_rows", [KR, HIDDEN], dt.bfloat16,
                            kind="ExternalInput").ap()
    xself = nc.dram_tensor("xself", [NBLK, P, HIDDEN], dt.bfloat16,
                           kind="ExternalInput").ap()
    sid = nc.dram_tensor("sid", [P, sumT], dt.int32,
                         kind="ExternalInput").ap()
    did = nc.dram_tensor("did", [P, sumT], dt.float32,
                         kind="ExternalInput").ap()
    colidx = nc.dram_tensor("colidx", [P, P], dt.float32,
                            kind="ExternalInput").ap()
    w1p = nc.dram_tensor("w1p", [P, M1, KSUB1, P], dt.bfloat16,
                         kind="ExternalInput").ap()
    b1p = nc.dram_tensor("b1p", [P, M1], dt.float32,
                         kind="ExternalInput").ap()
    w2p = nc.dram_tensor("w2p", [P, M2, M1, P], dt.bfloat16,
                         kind="ExternalInput").ap()
    b2p = nc.dram_tensor("b2p", [P, M2], dt.float32,
                         kind="ExternalInput").ap()
    out_d = nc.dram_tensor("out_d", [P, M2, ND], dt.float32,
                           kind="ExternalOutput").ap()

    with tile.TileContext(nc) as tc:
        with ExitStack() as ctx:
            const = ctx.enter_context(tc.tile_pool(name="const", bufs=1))
            st = {
                "x_rows": x_rows, "xself": xself, "w1p": w1p, "w2p": w2p,
                "out_d": out_d,
                "hp": ctx.enter_context(tc.tile_pool(name="hp", bufs=2)),
                "h1p": ctx.enter_context(tc.tile_pool(name="h1p", bufs=1)),
                "gp": ctx.enter_context(tc.tile_pool(name="gp", bufs=48)),
                "ohp": ctx.enter_context(tc.tile_pool(name="ohp", bufs=4)),
                "xbp": ctx.enter_context(tc.tile_pool(name="xbp", bufs=2)),
                "hsbp": ctx.enter_context(tc.tile_pool(name="hsbp", bufs=2)),
                "w1p_sb": ctx.enter_context(tc.tile_pool(name="w1sb", bufs=3)),
                "w2p_sb": ctx.enter_context(tc.tile_pool(name="w2sb", bufs=3)),
                "evp": ctx.enter_context(tc.tile_pool(name="evp", bufs=3)),
                "aggps": ctx.enter_context(
                    tc.tile_pool(name="aggps", bufs=2, space="PSUM")),
                "tpps": ctx.enter_context(
                    tc.tile_pool(name="tpps", bufs=1, space="PSUM")),
                "mmps": ctx.enter_context(
                    tc.tile_pool(name="mmps", bufs=4, space="PSUM")),
            }
            colidx_sb = const.tile([P, P], dt.float32)
            nc.sync.dma_start(colidx_sb[:], colidx[:])
            ident = const.tile([P, P], dt.float32)
            make_identity(nc, ident[:])
            sid_sb = const.tile([P, sumT], dt.int32)
            nc.sync.dma_start(sid_sb[:], sid[:])
            did_sb = const.tile([P, sumT], dt.float32)
            nc.sync.dma_start(did_sb[:], did[:])
            b1_sb = const.tile([P, M1], dt.float32)
            nc.sync.dma_start(b1_sb[:], b1p[:])
            b2_sb = const.tile([P, M2], dt.float32)
            nc.sync.dma_start(b2_sb[:], b2p[:])
            st.update(colidx_sb=colidx_sb, ident=ident, sid_sb=sid_sb,
                      did_sb=did_sb, b1_sb=b1_sb, b2_sb=b2_sb)

            max_nb = max(s[1] for s in SLICES)

            def new_h():
                return st["hp"].tile([P, KSUB1, max_nb * P], dt.bfloat16,
                                     name="h")

            def new_h1():
                return st["h1p"].tile([P, M1, max_nb * P], dt.bfloat16,
                                      name="h1")

            def emit_slice_A(si, h_tile):
                b0, nb, _ = SLICES[si]
                for bloc in range(nb):
                    b = b0 + bloc
                    _emit_ablock(nc, ctx, st, b, bloc, h_tile, Tb,
                                 int(offs[b]))

            # prologue: aggregate slice 0
            h_cur = new_h()
            emit_slice_A(0, h_cur)
            coffs = np.concatenate(
                [[0], np.cumsum([s[2] for s in SLICES])]).astype(int)
            for si in range(len(SLICES)):
                _, _, width = SLICES[si]
                chunks = _chunks(width)
                h1_tile = new_h1()
                _emit_mm1(nc, st, h_cur, h1_tile, chunks)
                # interleave next slice's aggregation into this slice's mm2
                if si + 1 < len(SLICES):
                    h_next = new_h()
                    nb_next = SLICES[si + 1][1]
                    spacing = M2 // nb_next
                    apos = {(j + 1) * spacing - spacing // 2: j
                            for j in range(nb_next)}
                else:
                    h_next = None
                    apos = {}
                for m in range(M2):
                    _emit_bunit(nc, st, h1_tile, m, chunks, int(coffs[si]))
                    if m in apos:
                        bloc = apos[m]
                        b = SLICES[si + 1][0] + bloc
                        _emit_ablock(nc, ctx, st, b, bloc, h_next, Tb,
                                     int(offs[b]))
                h_cur = h_next
    nc.compile()
    _BUILT[key] = nc
    return nc


def host_pack(x, edge_index, W1, b1, W2, b2):
    """Returns (in_maps per core, Tb tuple)."""
    x = np.asarray(x, dtype=np.float32)
    edge_index = np.asarray(edge_index)
    W1 = np.asarray(W1, dtype=np.float32)
    b1 = np.asarray(b1, dtype=np.float32)
    W2 = np.asarray(W2, dtype=np.float32)
    b2 = np.asarray(b2, dtype=np.float32)
    src = edge_index[0].astype(np.int64)
    dst = edge_index[1].astype(np.int64)

    core = dst // ND
    local = dst % ND
    blk = local // P
    within = (local % P).astype(np.float32)
    bucket = core * NBLK + blk
    counts = np.bincount(bucket, minlength=NCORES * NBLK).reshape(NCORES, NBLK)
    Tb = tuple(int(-(-counts[:, b].max() // P)) for b in range(NBLK))
    offs = np.concatenate([[0], np.cumsum(Tb)]).astype(np.int64)
    sumT = int(sum(Tb))

    order = np.argsort(bucket, kind="stable")
    bs = bucket[order]
    starts = np.zeros(NCORES * NBLK, dtype=np.int64)
    np.cumsum(counts.reshape(-1)[:-1], out=starts[1:])
    pos = np.arange(bs.size, dtype=np.int64) - starts[bs]
    ecore = bs // NBLK
    eblk = bs % NBLK
    t = pos // P
    prt = pos % P
    col = offs[eblk] + t

    sid_arr = np.full((NCORES, P, sumT), ZERO_ROW, dtype=np.int32)
    did_arr = np.zeros((NCORES, P, sumT), dtype=np.float32)
    sid_arr[ecore, prt, col] = src[order].astype(np.int32)
    did_arr[ecore, prt, col] = within[order]

    x_rows = np.zeros((KR, HIDDEN), dtype=BF16)
    x_rows[:N_NODES] = x
    xp = np.zeros((NCORES, NBLK * P, HIDDEN), dtype=BF16)
    xp[:, :ND] = x.reshape(NCORES, ND, HIDDEN)
    xp = xp.reshape(NCORES, NBLK, P, HIDDEN)

    colidx = np.ascontiguousarray(
        np.broadcast_to(np.arange(P, dtype=np.float32)[None, :], (P, P)))
    w1p = np.ascontiguousarray(
        W1.astype(BF16).reshape(M1, P, KSUB1, P).transpose(3, 0, 2, 1))
    w2p = np.ascontiguousarray(
        W2.astype(BF16).reshape(M2, P, M1, P).transpose(3, 0, 2, 1))
    b1p = np.ascontiguousarray(b1.reshape(M1, P).T).astype(np.float32)
    b2p = np.ascontiguousarray(b2.reshape(M2, P).T).astype(np.float32)

    in_maps = []
    for c in range(NCORES):
        in_maps.append({
            "x_rows": x_rows,
            "xself": np.ascontiguousarray(xp[c]),
            "sid": np.ascontiguousarray(sid_arr[c]),
            "did": np.ascontiguousarray(did_arr[c]),
            "colidx": colidx,
            "w1p": w1p,
            "b1p": b1p,
            "w2p": w2p,
            "b2p": b2p,
        })
    return in_maps, Tb


def build_args(in_maps_and_tb):
    return (in_maps_and_tb[1],)


def _make_runner(Tb):
    rkey = ("runner", Tb)
    if rkey in _BUILT:
        return _BUILT[rkey]
    import jax
    from jax.experimental.shard_map import shard_map
    from jax.sharding import Mesh, NamedSharding, PartitionSpec
    from concourse import bass2jax, mybir

    nc = _build(Tb)
    bass2jax.install_neuronx_cc_hook()

    pid_name = (nc.partition_id_tensor.name
                if nc.partition_id_tensor is not None else None)
    in_names, out_names, out_avals = [], [], []
    for alloc in nc.m.functions[0].allocations:
        if not isinstance(alloc, mybir.MemoryLocationSet):
            continue
        name = alloc.memorylocations[0].name
        if alloc.kind == "ExternalInput":
            if name != pid_name:
                in_names.append(name)
        elif alloc.kind == "ExternalOutput":
            out_names.append(name)
            out_avals.append(jax.core.ShapedArray(
                tuple(alloc.tensor_shape), mybir.dt.np(alloc.dtype)))
    n_params = len(in_names)
    all_names = in_names + out_names
    if pid_name is not None:
        all_names = all_names + [pid_name]
    donate = tuple(range(n_params, n_params + len(out_names)))

    def _body(*args):
        operands = list(args)
        if pid_name is not None:
            operands.append(bass2jax.partition_id_tensor())
        outs = bass2jax._bass_exec_p.bind(
            *operands,
            out_avals=tuple(out_avals),
            in_names=tuple(all_names),
            out_names=tuple(out_names),
            lowering_input_output_aliases=(),
            sim_require_finite=True,
            sim_require_nnan=True,
            nc=nc,
        )
        return tuple(outs)

    devices = jax.devices()[:NCORES]
    mesh = Mesh(np.asarray(devices), ("core",))
    spec = PartitionSpec("core")
    fn = jax.jit(
        shard_map(_body, mesh=mesh,
                  in_specs=(spec,) * (n_params + len(out_names)),
                  out_specs=(spec,) * len(out_names), check_rep=False),
        donate_argnums=donate, keep_unused=True)
    sharding = NamedSharding(mesh, spec)
    runner = dict(fn=fn, in_names=in_names, out_names=out_names,
                  out_avals=out_avals, sharding=sharding, mesh=mesh)
    _BUILT[rkey] = runner
    return runner


def _prep_device_inputs(in_maps, Tb):
    import jax
    r = _make_runner(Tb)
    concat = [np.concatenate([m[name] for m in in_maps], axis=0)
              for name in r["in_names"]]
    ins_dev = [jax.device_put(a, r["sharding"]) for a in concat]
    zeros = [
        jax.jit(lambda a=av: jax.numpy.zeros(
            (NCORES * a.shape[0], *a.shape[1:]), a.dtype),
            out_shardings=r["sharding"])()
        for av in r["out_avals"]
    ]
    jax.block_until_ready(ins_dev + zeros)
    return ins_dev, zeros


def _run_once(ins_dev, out_bufs, Tb):
    import jax
    r = _make_runner(Tb)
    outs = r["fn"](*ins_dev, *out_bufs)
    jax.block_until_ready(outs)
    return outs


def kernel(x, edge_index, W1, b1, W2, b2):
    global LAST_RESULTS
    in_maps, Tb = host_pack(x, edge_index, W1, b1, W2, b2)
    ins_dev, zeros = _prep_device_inputs(in_maps, Tb)
    outs = _run_once(ins_dev, zeros, Tb)
    LAST_RESULTS = dict(ins_dev=ins_dev, outs=outs, T=Tb)

    r = _make_runner(Tb)
    out_global = np.asarray(outs[r["out_names"].index("out_d")])
    out_global = out_global.reshape(NCORES, P, M2, ND)
    # out_global[c, p, m, n] = out[c*ND + n, m*128 + p]
    out = np.ascontiguousarray(
        out_global.transpose(0, 3, 2, 1)).reshape(N_NODES, VOCAB)
    return out


def bench(iters=5):
    import time
    st = LAST_RESULTS
    assert st is not None, "run kernel() first"
    outs = st["outs"]
    times = []
    for _ in range(iters):
        t0 = time.perf_counter()
        outs = _run_once(st["ins_dev"], outs, st["T"])
        times.append(time.perf_counter() - t0)
    st["outs"] = outs
    return times


def bench_pipelined(iters=24, base=3):
    """Per-iter device time from the slope between a `base`-call chain and a
    `base+iters`-call chain (the first-call dispatch overhead cancels)."""
    import time
    import jax
    st = LAST_RESULTS
    assert st is not None, "run kernel() first"
    r = _make_runner(st["T"])
    outs = st["outs"]
    outs = _run_once(st["ins_dev"], outs, st["T"])
    t0 = time.perf_counter()
    for _ in range(base):
        outs = r["fn"](*st["ins_dev"], *outs)
    jax.block_until_ready(outs)
    tA = time.perf_counter() - t0
    t0 = time.perf_counter()
    for _ in range(base + iters):
        outs = r["fn"](*st["ins_dev"], *outs)
    jax.block_until_ready(outs)
    tB = time.perf_counter() - t0
    st["outs"] = outs
    per_iter = (tB - tA) / iters
    return dict(t1=tA, tN=tB, iters=iters, per_iter=per_iter)
